# revision 26
# baseline (speedup 1.0000x reference)
"""Trainium2 Bass kernel for quantized BasicBlock (DoReFa conv-bn-quant x2 + skip).

Strategy:
- Data-parallel over batch: 128 images -> 16 per core across 8 cores.
- Weights quantize on-device to odd integers in [-15,15] (exact in bf16);
  the 1/15 (conv1) and 1/225 (conv2) scales fold into the BN affines.
- conv1: x split hi/lo into two bf16 tensors (products with 4-bit integer
  weights are exact at the PE's FP22 multiply precision).
- conv2: activations are 4-bit ints 0..15 (exact bf16) -> conv2 is exact
  integer arithmetic accumulated in fp32 PSUM.
- 3x3 conv with 2 pruned taps = 7 shifted matmuls [K=128,M=128,N=512]
  accumulated in PSUM over a zero-padded [C,34,34] SBUF image.
- Rounding via the +2^23 magic-add trick = IEEE RNE, matching jnp.round.
"""
import numpy as np

import concourse.bass as bass
import concourse.tile as tile
from concourse import bacc, mybir, masks
from concourse.bass_utils import run_bass_kernel_spmd

AF = mybir.ActivationFunctionType
OP = mybir.AluOpType
F32 = mybir.dt.float32
BF16 = mybir.dt.bfloat16

B, C, H, W = 128, 128, 32, 32
NCORES = 8
BL = B // NCORES          # images per core
HP, WP = H + 2, W + 2     # zero-padded image
NPIX = H * W
TAPS = [(0, 1), (0, 2), (1, 0), (1, 1), (1, 2), (2, 0), (2, 1)]  # (0,0),(2,2) pruned
MAGIC = float(2 ** 23)
EPS = 1e-5
NB = 2                    # padded-buffer pipeline depth
SPOOL_BUFS = 2
IPOOL_BUFS = 3
CONV1_MODE = "split2"     # "split2" (2x bf16 hi/lo, ~8e-6 err) | "f32r" (1x fp22, ~1e-4 err)

BN_NAMES = ["gamma1", "beta1", "mean1", "var1", "gamma2", "beta2", "mean2", "var2"]


def _emit_weight_quant(tc, pool, psum_pool, ident, w_dram, wT, tmp, wT2=None):
    """Quantize w (DRAM [C,C,3,3]) to integer taps, transposed: wT[i, tap*C+o] bf16."""
    nc = tc.nc
    wsb = pool.tile([C, C, 9], F32, tag="wq_wsb")
    nc.sync.dma_start(wsb[:], w_dram)
    tw = pool.tile([C, C, 9], F32, tag="wq_tw")
    nc.scalar.activation(tw[:], wsb[:], AF.Tanh)
    am = pool.tile([C, 1], F32, tag="wq_am")
    nc.vector.tensor_reduce(am[:], tw[:], axis=mybir.AxisListType.XY,
                            op=OP.max, apply_absolute_value=True)
    # cross-partition max: transpose [128,1] -> [1,128], reduce, broadcast back
    tpm = psum_pool.tile([1, C], F32, tag="ps")
    nc.tensor.transpose(tpm[:], am[:], ident[:])
    mx = pool.tile([1, 1], F32, tag="wq_mx")
    nc.vector.reduce_max(mx[:], tpm[:], axis=mybir.AxisListType.X)
    bps = psum_pool.tile([C, 1], F32, tag="ps")
    nc.tensor.matmul(bps[:], tmp["ones_row"][:], mx[:], start=True, stop=True)
    mb = pool.tile([C, 1], F32, tag="wq_mb")
    nc.vector.tensor_copy(mb[:], bps[:])
    rec = pool.tile([C, 1], F32, tag="wq_rec")
    nc.vector.reciprocal(rec[:], mb[:])
    c150 = pool.tile([C, 1], F32, tag="wq_c150")
    nc.vector.tensor_scalar_mul(c150[:], rec[:], 15.0)
    # u2 = 2u = tanh * (15/M) + 15 in [0,30]; round-to-even-multiple via 2^24
    # magic gives 2*round(u) exactly; -15 folds into the PSUM->SBUF copy.
    u2 = pool.tile([C, C, 9], F32, tag="wq_u2")
    nc.scalar.activation(u2[:], tw[:], AF.Identity, bias=tmp["b15"][:], scale=c150[:])
    wint = pool.tile([C, C, 9], F32, tag="wq_wint")
    nc.vector.tensor_scalar(wint[:], u2[:], 2.0 * MAGIC, 2.0 * MAGIC, OP.add, OP.subtract)
    for ti, (ky, kx) in enumerate(TAPS):
        t = ky * 3 + kx
        tp = psum_pool.tile([C, C], F32, tag="ps")
        nc.tensor.transpose(tp[:], wint[:, :, t], ident[:])
        nc.vector.tensor_scalar(wT[:, ti * C:(ti + 1) * C], tp[:], 15.0, None, OP.subtract)
        if wT2 is not None:
            nc.vector.tensor_scalar(wT2[:, ti * C:(ti + 1) * C], tp[:], 15.0, None,
                                    OP.subtract)


def _emit_rsqrt(nc, pool, var, name):
    """1/sqrt(var+eps), ACT-sqrt seed + 2 Newton steps (ACT sqrt is low-precision)."""
    veps = pool.tile([C, 1], F32, tag=f"{name}_veps")
    nc.vector.tensor_scalar_add(veps[:], var[:], EPS)
    sq = pool.tile([C, 1], F32, tag=f"{name}_sq")
    nc.scalar.activation(sq[:], veps[:], AF.Sqrt)
    y = pool.tile([C, 1], F32, tag=f"{name}_y")
    nc.vector.reciprocal(y[:], sq[:])
    c15 = pool.tile([C, 1], F32, tag=f"{name}_c15")
    nc.vector.memset(c15[:], 1.5)
    for it in range(2):
        a = pool.tile([C, 1], F32, tag=f"{name}_a{it}")
        nc.vector.tensor_mul(a[:], y[:], y[:])
        nc.vector.tensor_mul(a[:], a[:], veps[:])
        d = pool.tile([C, 1], F32, tag=f"{name}_d{it}")
        nc.vector.scalar_tensor_tensor(d[:], a[:], -0.5, c15[:], OP.mult, OP.add)
        y2 = pool.tile([C, 1], F32, tag=f"{name}_y{it}")
        nc.vector.tensor_mul(y2[:], y[:], d[:])
        y = y2
    return y


def _emit(tc, dr, bl, repeat=1):
    nc = tc.nc
    with tc.tile_pool(name="const", bufs=1) as cpool, \
         tc.tile_pool(name="img", bufs=IPOOL_BUFS) as ipool, \
         tc.tile_pool(name="stage", bufs=SPOOL_BUFS) as spool, \
         tc.tile_pool(name="ps1", bufs=2, space="PSUM") as pp1, \
         tc.tile_pool(name="ps2", bufs=2, space="PSUM") as pp2:

        ident = cpool.tile([C, C], F32, tag="ident")
        masks.make_identity(nc, ident[:])
        ones_row = cpool.tile([1, C], F32, tag="ones_row")
        nc.vector.memset(ones_row[:], 1.0)
        b15 = cpool.tile([C, 1], F32, tag="b15")
        nc.vector.memset(b15[:], 15.0)
        tmp = {"ones_row": ones_row, "b15": b15}

        w1T = cpool.tile([C, 7 * C],
                         mybir.dt.float32r if CONV1_MODE == "f32r" else BF16, tag="w1T")
        w2T = cpool.tile([C, 7 * C], BF16, tag="w2T")
        # fp16 copy of conv1 weights for the lo-residual pass (ints exact in fp16)
        w1Th = (cpool.tile([C, 7 * C], mybir.dt.float16, tag="w1Th", name="w1Th")
                if CONV1_MODE == "split2" else None)
        _emit_weight_quant(tc, cpool, pp1, ident, dr["w1"], w1T, tmp, wT2=w1Th)
        _emit_weight_quant(tc, cpool, pp1, ident, dr["w2"], w2T, tmp)

        # BN affines (scales/biases on the x15 integer grid)
        bn = {}
        for nm in BN_NAMES:
            v = cpool.tile([C, 1], F32, tag=f"bn_{nm}")
            nc.sync.dma_start(v[:], dr[nm])
            bn[nm] = v
        rs1 = _emit_rsqrt(nc, cpool, bn["var1"], "rs1")
        rs2 = _emit_rsqrt(nc, cpool, bn["var2"], "rs2")
        inv1 = cpool.tile([C, 1], F32, tag="inv1")
        nc.vector.tensor_mul(inv1[:], bn["gamma1"][:], rs1[:])
        inv2 = cpool.tile([C, 1], F32, tag="inv2")
        nc.vector.tensor_mul(inv2[:], bn["gamma2"][:], rs2[:])
        sc2 = cpool.tile([C, 1], F32, tag="sc2")
        nc.vector.tensor_scalar_mul(sc2[:], inv2[:], 1.0 / 15.0)
        b_s = {}
        for k, invk in (("1", inv1), ("2", inv2)):
            mb_ = cpool.tile([C, 1], F32, tag=f"mb{k}")
            nc.vector.tensor_mul(mb_[:], bn[f"mean{k}"][:], invk[:])
            bsc = cpool.tile([C, 1], F32, tag=f"bsc{k}")
            nc.vector.tensor_scalar_mul(bsc[:], bn[f"beta{k}"][:], 15.0)
            bs = cpool.tile([C, 1], F32, tag=f"bs{k}")
            nc.vector.scalar_tensor_tensor(bs[:], mb_[:], -15.0, bsc[:], OP.mult, OP.add)
            b_s[k] = bs

        # persistent zero-padded image buffers (borders zeroed once)
        a1_t = [cpool.tile([C, HP, WP], BF16, tag=f"a1{k}", name=f"a1{k}") for k in range(NB)]
        if CONV1_MODE == "f32r":
            xp_t = [cpool.tile([C, HP, WP], mybir.dt.float32r, tag=f"xp{k}", name=f"xp{k}")
                    for k in range(NB)]
            for t in a1_t:
                nc.gpsimd.memset(t[:], 0.0)
            for t in xp_t:
                nc.gpsimd.memset(t[:].bitcast(F32), 0.0)
        else:
            xhi_t = [cpool.tile([C, HP, WP], BF16, tag=f"xhi{k}", name=f"xhi{k}") for k in range(NB)]
            xlo_t = [cpool.tile([C, HP, WP], mybir.dt.float16, tag=f"xlo{k}", name=f"xlo{k}")
                     for k in range(NB)]
            for t in xhi_t + xlo_t + a1_t:
                nc.gpsimd.memset(t[:], 0.0)

        F32R = mybir.dt.float32r

        def _images():
            for i in range(bl):
                _image(i)

        def _image(i):
            a1 = a1_t[i % NB]
            a1_in = a1[:, 1:H + 1, 1:W + 1]

            # load x; build conv1 operands
            if CONV1_MODE == "f32r":
                xp = xp_t[i % NB]
                xsb = ipool.tile([C, H, W], F32, tag="xsb")
                nc.sync.dma_start(xsb[:], dr["x"][i])
                nc.scalar.activation(xp[:, 1:H + 1, 1:W + 1], xsb[:], AF.Copy)
                x_skip = xsb[:]
            else:
                xhi, xlo = xhi_t[i % NB], xlo_t[i % NB]
                xhi_in = xhi[:, 1:H + 1, 1:W + 1]
                xlo_in = xlo[:, 1:H + 1, 1:W + 1]
                xsb = ipool.tile([C, H, W], F32, tag="xsb")
                nc.sync.dma_start(xsb[:], dr["x"][i])
                nc.scalar.activation(xhi_in, xsb[:], AF.Copy)      # bf16 cast (hi)
                nc.vector.scalar_tensor_tensor(xlo_in, xhi_in, -1.0, xsb[:],
                                               OP.mult, OP.add)    # lo = x - hi
                x_skip = xsb[:]

            # conv1: accumulate 7 taps (x {hi,lo} in split2) per 512-pixel half
            ps1 = pp1.tile([C, NPIX], F32, tag="ps")
            for h in (0, 1):
                out_ap = ps1[:, h * 512:(h + 1) * 512]
                for ti, (ky, kx) in enumerate(TAPS):
                    wtap = w1T[:, ti * C:(ti + 1) * C]
                    r0 = 16 * h + ky
                    if CONV1_MODE == "f32r":
                        nc.tensor.matmul(out_ap, wtap,
                                         xp[:, r0:r0 + 16, kx:kx + W],
                                         start=(ti == 0), stop=(ti == len(TAPS) - 1))
                    else:
                        nc.tensor.matmul(out_ap, wtap, xhi[:, r0:r0 + 16, kx:kx + W],
                                         start=(ti == 0), stop=False)
                        nc.tensor.matmul(out_ap, w1Th[:, ti * C:(ti + 1) * C],
                                         xlo[:, r0:r0 + 16, kx:kx + W],
                                         start=False, stop=(ti == len(TAPS) - 1))

            # stage1: a1 = round(clip(s1*inv1 + 15*b1, 0, 15))  (ints 0..15, bf16)
            ps1_3 = ps1[:].rearrange("c (h w) -> c h w", h=H)
            r = spool.tile([C, H, W], F32, tag="st_r")
            nc.scalar.activation(r[:], ps1_3, AF.Relu, bias=b_s["1"][:], scale=inv1[:])
            q = spool.tile([C, H, W], F32, tag="st_q")
            nc.vector.tensor_scalar(q[:], r[:], 15.0, MAGIC, OP.min, OP.add)
            nc.vector.tensor_scalar(a1_in, q[:], MAGIC, None, OP.subtract)

            # conv2: exact integer conv on a1
            ps2 = pp2.tile([C, NPIX], F32, tag="ps")
            for h in (0, 1):
                out_ap = ps2[:, h * 512:(h + 1) * 512]
                for ti, (ky, kx) in enumerate(TAPS):
                    wtap = w2T[:, ti * C:(ti + 1) * C]
                    r0 = 16 * h + ky
                    nc.tensor.matmul(out_ap, wtap, a1[:, r0:r0 + 16, kx:kx + W],
                                     start=(ti == 0), stop=(ti == len(TAPS) - 1))

            # stage2: out = round(clip(s2*inv2/15 + 15*b2 + 15*x, 0, 15)) / 15
            ps2_3 = ps2[:].rearrange("c (h w) -> c h w", h=H)
            g = spool.tile([C, H, W], F32, tag="st_g")
            nc.scalar.activation(g[:], ps2_3, AF.Identity, bias=b_s["2"][:], scale=sc2[:])
            hh = spool.tile([C, H, W], F32, tag="st_h")
            nc.vector.scalar_tensor_tensor(hh[:], x_skip, 15.0, g[:], OP.mult, OP.add)
            p = spool.tile([C, H, W], F32, tag="st_p")
            nc.vector.tensor_scalar(p[:], hh[:], 0.0, MAGIC, OP.max, OP.add)
            t = spool.tile([C, H, W], F32, tag="st_t")
            nc.vector.tensor_scalar(t[:], p[:], MAGIC, 15.0, OP.subtract, OP.min)
            ob = spool.tile([C, H, W], F32, tag="st_ob")
            nc.scalar.activation(ob[:], t[:], AF.Copy, scale=1.0 / 15.0)
            nc.sync.dma_start(dr["y"][i], ob[:])

        if repeat > 1:
            with tc.For_i(0, repeat, 1):
                _images()
        else:
            _images()


def _build(bl=BL, repeat=1):
    nc = bacc.Bacc("TRN2", target_bir_lowering=False, debug=False,
                   enable_asserts=False, num_devices=NCORES)
    dr = {}
    dr["x"] = nc.dram_tensor("x", [bl, C, H, W], F32, kind="ExternalInput").ap()
    dr["w1"] = nc.dram_tensor("w1", [C, C, 9], F32, kind="ExternalInput").ap()
    dr["w2"] = nc.dram_tensor("w2", [C, C, 9], F32, kind="ExternalInput").ap()
    for nm in BN_NAMES:
        dr[nm] = nc.dram_tensor(nm, [C, 1], F32, kind="ExternalInput").ap()
    dr["y"] = nc.dram_tensor("y", [bl, C, H, W], F32, kind="ExternalOutput").ap()
    with tile.TileContext(nc) as tc:
        _emit(tc, dr, bl, repeat=repeat)
    nc.compile()
    return nc


_CACHED = None


def _in_maps(inputs, bl=BL, ncores=NCORES):
    f = lambda v: np.ascontiguousarray(np.asarray(v, dtype=np.float32))
    x = f(inputs["x"])
    base = {"w1": f(inputs["w1"]).reshape(C, C, 9),
            "w2": f(inputs["w2"]).reshape(C, C, 9)}
    for nm in BN_NAMES:
        base[nm] = f(inputs[nm]).reshape(C, 1)
    maps = []
    for c in range(ncores):
        m = dict(base)
        m["x"] = np.ascontiguousarray(x[c * bl:(c + 1) * bl])
        maps.append(m)
    return maps


def _run(inputs, trace=False):
    global _CACHED
    if _CACHED is None:
        _CACHED = _build()
    res = run_bass_kernel_spmd(_CACHED, _in_maps(inputs),
                               core_ids=list(range(NCORES)), trace=trace)
    y = np.concatenate([res.results[c]["y"] for c in range(NCORES)], axis=0)
    return y.astype(np.float32), res


def kernel(**inputs) -> np.ndarray:
    y, _ = _run(inputs, trace=False)
    return y
